# revision 44
# baseline (speedup 1.0000x reference)
"""DFA-GNN (max-aggregation message passing) Trainium2 kernel.

Problem (B=2, N=4096, E=65536, M=4, H=256), per batch b:
    coeff[e]  = edge_fts[b,e,:] @ edge_W + edge_b                  # [E]
    agg[n]    = max over edges e with tgt[e]==n of coeff[e] * hint[b, src[e]]
    out[b,n]  = (node_fts[b,n] + agg[n]) @ update_W + update_b     # [M,H] rows

Sharding: 8 cores = 2 batches x 4 target-node quarters (1024 nodes each).
Edges are bucketed by target node on the host (every node has exactly 16
incoming edges with this generator; general counts <=16 are padded by
duplicating an edge, which preserves the max).

The gather is the whole game: 16384 hint rows per core.  SWDGE descriptor
emission on the gpsimd Q7 costs ~8-11 ns per gathered row, so the f32
baseline (128 indirect DMAs x 128 rows) was instruction-rate-bound at
~2.1 us/instruction (~265 us of gather span).  This kernel instead:

  - stores hint/edge features in bf16 (harness gate is 2e-2; bf16 lands
    ~4e-3), halving HBM traffic,
  - uses bulk `dma_gather` ucode (one instruction per 512-1024 rows),
  - optionally (KERNEL_RPD=2, default) packs TWO hint rows per descriptor:
    the host lays the hint table out as Eulerian trails over the per-node
    source pairs, so a single 4 KB window (elem_step = one row) covers one
    PAIR of edges of one node.  Descriptor count halves (8192/core).  This
    is index preprocessing only - the table is ~1x the hint size plus trail
    breaks; messages are never materialized on the host.
  - computes edge coefficients with a 2-block lookahead (PE matmuls with
    edge_W stationary; edge features stream on SWDGE queue 0, gathers on
    queue 1 so the SDMA engines round-robin between them),
  - multiplies gathered rows by per-edge coefficients on the scalar engine
    (activation Copy with per-partition scale) plus a few DVE tensor_scalar
    ops (4x packed mode) - scalar_tensor_tensor has no 2x uop so the fused
    mult+max chain would run at 1 elem/cycle/lane,
  - max-reduces with double-width (two rank lanes) DVE tensor_tensor ops in
    2x packed bf16 mode, folding the two lanes at the end,
  - folds +node_fts on DVE, transposes via PE, applies update_W as PE
    matmuls with +update_b as a rank-1 seed term; output is stored bf16 and
    upcast on the host.

Each block's epilogue (lane fold, +node_fts, PE transposes, update_W
matmuls, output copy/DMA) is emitted one block late: engines run their
instruction streams in trace order, so an epilogue op that waits on the PE
pipeline would otherwise stall the engine ahead of the next block's
multiplies (~4.5 us/block recovered).

Per-core HBM: 32 MB gather + 8.4 MB edge features + 2 MB node features
+ 2 MB bf16 output ~= 44 MB -> ~125 us floor at 358 GB/s; measured
185-198 us (~129 us scalar-engine busy, ~119 us DVE, ~100 us gpsimd,
~40 us head before the first gather tile lands).
"""

import os
import sys

import numpy as np

for _p in ("/opt/trn_rl_repo", "/root/.axon_site/_ro/trn_rl_repo"):
    if os.path.isdir(_p) and _p not in sys.path:
        sys.path.insert(0, _p)

B, N, E, M, H = 2, 4096, 65536, 4, 256
MH = M * H            # 1024
P = 128               # partitions
K = 16                # edges per node (E // N)
NCORE = N // 4        # nodes per core (1024)
NB = NCORE // P       # node blocks per core (8)
EC = NCORE * K        # edges per core (16384)
N_CORES = 8

# rows (hint vectors) per gather descriptor: 1 = plain gather, 2 = pair
# windows over the Eulerian-trail table.
RPD = int(os.environ.get("KERNEL_RPD", "2"))
assert RPD in (1, 2)
# gather tile = 8 edge ranks (half a block): [P, 8*MH] elems.
RANKS_PT = 8                       # edge ranks per gather tile
WINS_PB = K // RPD                 # windows per node per block
WINS_PT = RANKS_PT // RPD          # windows per node per tile
NIDX = P * WINS_PT                 # indices per dma_gather instruction
GI_PER_CORE = NB * 2               # gather instructions per core
NWIN_CORE = EC // RPD              # gather windows per core
# trail table rows: EC/2 pair-edges + one extra row per trail break.
TMAX = N if RPD == 1 else (EC // RPD + 2048)
GT_BUFS = int(os.environ.get("KERNEL_GT_BUFS", "5"))
# edge ranks whose coeff-multiply runs on the scalar engine (rest on DVE)
ACT_RANKS = int(os.environ.get("KERNEL_ACT_RANKS", "9"))

_CACHE = {}

# Set by kernel() when KERNEL_TRACE=1: BassKernelResults of the last run.
LAST_RESULT = None


def _build(rpd, bf16):
    from concourse import bass, bacc, mybir, tile
    from concourse.ap import AP
    from concourse.masks import make_identity

    f32 = mybir.dt.float32
    i16 = mybir.dt.int16
    gdt = mybir.dt.bfloat16 if bf16 else f32
    ECB = P * K  # edge-feature columns per node block (2048)

    nc = bacc.Bacc("TRN2", target_bir_lowering=False, debug=False,
                   num_devices=N_CORES, num_swdge_queues=2)

    table = nc.dram_tensor("table", [TMAX, MH], gdt, kind="ExternalInput")
    eftsT = nc.dram_tensor("eftsT", [H, EC], gdt, kind="ExternalInput")
    idx_d = nc.dram_tensor("idx16", [P, NWIN_CORE // 16], i16,
                           kind="ExternalInput")
    nf_d = nc.dram_tensor("nf", [NCORE, MH], gdt, kind="ExternalInput")
    eW_d = nc.dram_tensor("eW", [P, 2], gdt, kind="ExternalInput")
    eb_d = nc.dram_tensor("eb", [P, 1], f32, kind="ExternalInput")
    uW_d = nc.dram_tensor("uW", [H, H], gdt, kind="ExternalInput")
    ub_d = nc.dram_tensor("ub", [1, MH], gdt, kind="ExternalInput")
    out_d = nc.dram_tensor("out", [NCORE, MH], gdt, kind="ExternalOutput")

    with tile.TileContext(nc) as tc:
        from concourse.mybir import AluOpType as alu

        with (
            tc.tile_pool(name="const", bufs=1) as cpool,
            tc.tile_pool(name="efts", bufs=3) as epool,
            tc.tile_pool(name="gt", bufs=GT_BUFS) as gpool,
            tc.tile_pool(name="acc", bufs=3) as apool,
            tc.tile_pool(name="scaled", bufs=6) as spool,
            tc.tile_pool(name="work", bufs=2) as wpool,
            tc.tile_pool(name="ps_coeff", bufs=2, space="PSUM") as ps_coeff,
            tc.tile_pool(name="ps_xt", bufs=2, space="PSUM") as ps_xt,
            tc.tile_pool(name="ps_out", bufs=1, space="PSUM") as ps_out,
        ):
            # gather index table first: it gates the first dma_gather.
            idx_t = cpool.tile([P, NWIN_CORE // 16], i16)
            nc.sync.dma_start(out=idx_t[:], in_=idx_d[:])
            eW = cpool.tile([P, 2], gdt)
            nc.sync.dma_start(out=eW[:], in_=eW_d[:])
            eb = cpool.tile([P, 1], f32)
            nc.sync.dma_start(out=eb[:], in_=eb_d[:])
            uW0 = cpool.tile([P, H], gdt)
            uW1 = cpool.tile([P, H], gdt)
            nc.sync.dma_start(out=uW0[:], in_=uW_d[0:P, :])
            nc.sync.dma_start(out=uW1[:], in_=uW_d[P:2 * P, :])
            ub_row = cpool.tile([1, MH], gdt)
            nc.sync.dma_start(out=ub_row[:], in_=ub_d[:])
            # all node features prefetched in one DMA during the idle head
            nf_all = cpool.tile([P, NB, MH], gdt)
            nc.sync.dma_start(
                out=nf_all[:],
                in_=nf_d[:].rearrange("(nb p) f -> p nb f", p=P))

            # source AP for gathers: overlapping 2-row windows when rpd=2.
            if rpd == 2:
                tbl_ap = AP(table, 0, [[MH, TMAX - 1], [1, rpd * MH]])
                estep = MH
            else:
                tbl_ap = table[:]
                estep = None

            # ---- decoupled coefficient prologue (all blocks) ----
            # eftsT columns are node-major within each block:
            # col nb*2048 + p*16 + r  ->  edge rank r of node p.
            coeff = cpool.tile([P, NB * K], f32)

            def emit_coeff(nb):
                efts0 = epool.tile([P, ECB], gdt, tag="efts0")
                efts1 = epool.tile([P, ECB], gdt, tag="efts1")
                nc.gpsimd.dma_start(out=efts0[:],
                                    in_=eftsT[0:P, nb * ECB:(nb + 1) * ECB])
                nc.gpsimd.dma_start(out=efts1[:],
                                    in_=eftsT[P:2 * P, nb * ECB:(nb + 1) * ECB])
                co_ps = ps_coeff.tile([128, 1024], f32, tag="co_ps",
                                      space="PSUM")
                for c in range(4):
                    pp, ff = (c % 2) * 64, (c // 2) * 512
                    nc.tensor.matmul(co_ps[pp:pp + 1, ff:ff + 512],
                                     lhsT=eW[:, 0:1],
                                     rhs=efts0[:, c * 512:(c + 1) * 512],
                                     start=True, stop=False)
                    nc.tensor.matmul(co_ps[pp:pp + 1, ff:ff + 512],
                                     lhsT=eW[:, 1:2],
                                     rhs=efts1[:, c * 512:(c + 1) * 512],
                                     start=False, stop=True)
                co_row = wpool.tile([P, 512], f32, tag="co_row")
                for c in range(4):
                    pp, ff = (c % 2) * 64, (c // 2) * 512
                    nc.scalar.add(co_row[c * 32:c * 32 + 1, :],
                                  co_ps[pp:pp + 1, ff:ff + 512], eb[0:1, 0:1])
                for c in range(4):
                    nc.sync.dma_start(
                        out=coeff[c * 32:(c + 1) * 32,
                                  nb * K:(nb + 1) * K],
                        in_=co_row[c * 32:c * 32 + 1, :].rearrange(
                            "c (p k) -> c p k", k=K))

            def emit_gather(nb, h, queue=1):
                gt = gpool.tile([P, RANKS_PT * MH], gdt, tag="gt")
                c0 = (nb * 2 + h) * NIDX // 16
                nc.gpsimd.dma_gather(
                    gt[:].rearrange("p (g e) -> p g e", e=rpd * MH),
                    tbl_ap,
                    idx_t[:, c0:c0 + NIDX // 16],
                    NIDX, NIDX, rpd * MH,
                    elem_step=estep,
                    queue_num=queue,
                )
                return gt

            emit_coeff(0)
            emit_coeff(1)
            ident = cpool.tile([P, P], gdt)
            make_identity(nc, ident[:])
            ones1 = cpool.tile([1, P], gdt)
            nc.vector.memset(ones1[:], 1.0)

            # ---- gather + scale (ACT/DVE) + max (DVE 2x) per block ----
            # scalar_tensor_tensor has no 2x uop, so the fused mult+max chain
            # runs at 1 elem/cycle/lane; instead the multiply runs on the
            # scalar engine (activation Copy with per-partition scale) or as
            # DVE tensor_scalar (4x mode), and the max is a DVE
            # tensor_tensor (2x packed mode for bf16).
            from concourse.mybir import ActivationFunctionType as actfn
            pend1, pend2 = [], []

            def stage1(nb, acc2):
                # fold the two max lanes, add node_fts, transpose via PE
                acc = acc2[:, 0:MH]
                nc.vector.tensor_tensor(out=acc, in0=acc,
                                        in1=acc2[:, MH:2 * MH], op=alu.max)
                nc.vector.tensor_tensor(out=acc, in0=acc,
                                        in1=nf_all[:, nb, :], op=alu.add)
                xt_ps = ps_xt.tile([P, MH], gdt, tag="xt_ps", space="PSUM")
                for c in range(MH // P):
                    nc.tensor.matmul(xt_ps[:, c * P:(c + 1) * P],
                                     lhsT=acc2[:, c * P:(c + 1) * P],
                                     rhs=ident[:], is_transpose=True,
                                     start=True, stop=True)
                return nb, xt_ps

            def stage2(nb, xt_ps):
                # out = X @ update_W + 1 x update_b, store
                xt = wpool.tile([P, MH], gdt, tag="xt")
                nc.vector.tensor_scalar(out=xt[:], in0=xt_ps[:], scalar1=1.0,
                                        scalar2=None, op0=alu.mult)
                o_ps = ps_out.tile([P, MH], f32, tag="o_ps", space="PSUM")
                for m in range(M):
                    nc.tensor.matmul(o_ps[:, m * H:(m + 1) * H],
                                     lhsT=ones1[0:1, :],
                                     rhs=ub_row[0:1, m * H:(m + 1) * H],
                                     start=True, stop=False)
                    nc.tensor.matmul(o_ps[:, m * H:(m + 1) * H],
                                     lhsT=xt[:, (2 * m) * P:(2 * m + 1) * P],
                                     rhs=uW0[:], start=False, stop=False)
                    nc.tensor.matmul(o_ps[:, m * H:(m + 1) * H],
                                     lhsT=xt[:, (2 * m + 1) * P:(2 * m + 2) * P],
                                     rhs=uW1[:], start=False, stop=True)
                o = wpool.tile([P, MH], gdt, tag="o")
                nc.scalar.copy(o[:], o_ps[:])
                nc.sync.dma_start(out=out_d[nb * P:(nb + 1) * P, :], in_=o[:])

            for nb in range(NB):
                if nb + 2 < NB:
                    emit_coeff(nb + 2)
                # acc2 holds two parallel max lanes (even/odd rank pairs) so
                # the DVE max runs as half as many double-width ops.
                acc2 = apool.tile([P, 2 * MH], gdt, tag="acc2")
                for h in range(2):
                    gt = emit_gather(nb, h)
                    for pr in range(RANKS_PT // 2):
                        r0 = h * RANKS_PT + 2 * pr
                        if r0 == 0:
                            st = acc2
                        else:
                            st = spool.tile([P, 2 * MH], gdt, tag="scaled")
                        for half in range(2):
                            r = r0 + half
                            src = gt[:, (2 * pr + half) * MH:
                                     (2 * pr + half + 1) * MH]
                            sc = coeff[:, nb * K + r:nb * K + r + 1]
                            dst = st[:, half * MH:(half + 1) * MH]
                            if r < ACT_RANKS:
                                nc.scalar.activation(dst, src, actfn.Copy,
                                                     scale=sc)
                            else:
                                nc.vector.tensor_scalar(
                                    out=dst, in0=src, scalar1=sc,
                                    scalar2=None, op0=alu.mult)
                        if r0 > 0:
                            nc.vector.tensor_tensor(out=acc2[:], in0=acc2[:],
                                                    in1=st[:], op=alu.max)
                # epilogue stages are emitted 1 and 2 blocks late so their
                # cross-engine dependencies (PE transposes, out matmul) are
                # ready when ACT/DVE reach those ops in-stream.
                pend1.append((nb, acc2))
                if len(pend2) > 1:
                    stage2(*pend2.pop(0))
                if len(pend1) > 1:
                    pend2.append(stage1(*pend1.pop(0)))
            pend2.append(stage1(*pend1.pop(0)))
            while pend2:
                stage2(*pend2.pop(0))

    nc.compile()
    return nc


def _install_ntff_hook():
    """Register the axon NTFF profiling hook if this image's antenv lacks it.

    Mirrors what trn_boot does when ``antenv.axon_hooks`` exists. Safe no-op
    on failure - tracing is skipped, execution still works.
    """
    import types

    try:
        import antenv.axon_hooks  # noqa: F401
        return
    except ImportError:
        pass
    try:
        import antenv
        from trn_agent_boot.trn_boot import _ntff_profile_via_ctypes

        hook = _ntff_profile_via_ctypes("/opt/axon/libaxon_pjrt.so")
        mod = types.ModuleType("antenv.axon_hooks")
        state = {"hook": hook}
        mod.get_axon_ntff_profile_hook = lambda: state["hook"]
        mod.set_axon_ntff_profile_hook = lambda h: state.update(hook=h)
        sys.modules["antenv.axon_hooks"] = mod
        antenv.axon_hooks = mod
    except Exception as e:  # pragma: no cover - best effort
        print(f"ntff hook install failed: {e}", file=sys.stderr)


def _edge_grid(tgt_b):
    """[N, K] edge ids bucketed by target node, padded by duplication."""
    counts = np.bincount(tgt_b, minlength=N)
    if counts.max() > K or counts.min() < 1:
        raise ValueError(f"edge counts per node outside [1, {K}]: "
                         f"min={counts.min()} max={counts.max()}")
    order = np.argsort(tgt_b, kind="stable")
    if (counts == K).all():
        return order.reshape(N, K)
    pos = np.zeros(N + 1, np.int64)
    np.cumsum(counts, out=pos[1:])
    offs = np.minimum(np.arange(K)[None, :], (counts - 1)[:, None])
    return order[pos[:-1, None] + offs]


def _pair_trails(s_q, g_q):
    """Eulerian-trail packing of per-node source pairs.

    s_q, g_q: [NCORE, K] sources / edge ids in grid order.  Pairs are
    (rank 2j, 2j+1).  Returns (trail_rows, gidx, eids) where trail_rows
    is the table's hint-row sequence, gidx [NCORE, K//2] the window index
    per pair, and eids [NCORE, K] the edge id per final rank slot (pair
    members swapped when the trail traverses the pair backwards).
    """
    npairs = NCORE * (K // 2)
    pa = s_q[:, 0::2].reshape(-1)          # pair endpoint a
    pb = s_q[:, 1::2].reshape(-1)          # pair endpoint b
    # adjacency: for vertex v, list of (pair_id, other, is_second_endpoint)
    adj = [[] for _ in range(N)]
    for pid in range(npairs):
        a, b = int(pa[pid]), int(pb[pid])
        adj[a].append((pid, b, False))
        adj[b].append((pid, a, True))
    ptr = [0] * N
    used = np.zeros(npairs, bool)
    win = np.empty(npairs, np.int64)
    flip = np.zeros(npairs, bool)
    trail_rows = []

    def walk(start):
        v = start
        trail = [v]
        while True:
            lst = adj[v]
            i = ptr[v]
            nxt = None
            while i < len(lst):
                pid, other, second = lst[i]
                if not used[pid]:
                    nxt = (pid, other, second)
                    i += 1
                    break
                i += 1
            ptr[v] = i
            if nxt is None:
                break
            pid, other, second = nxt
            used[pid] = True
            win[pid] = len(trail_rows) + len(trail) - 1
            flip[pid] = second          # traversed b -> a
            trail.append(other)
            v = other
        return trail

    deg = np.bincount(np.concatenate([pa, pb]), minlength=N)
    order = np.argsort(-(deg % 2))      # odd-degree vertices first
    for v in order:
        v = int(v)
        while ptr[v] < len(adj[v]):
            if used[adj[v][ptr[v]][0]]:
                ptr[v] += 1
                continue
            trail_rows.extend(walk(v))
    assert used.all()

    gidx = win.reshape(NCORE, K // 2)
    fl = flip.reshape(NCORE, K // 2)
    eids = g_q.reshape(NCORE, K // 2, 2).copy()
    eids[fl] = eids[fl][:, ::-1]
    return np.asarray(trail_rows, np.int64), gidx, eids.reshape(NCORE, K)


def kernel(**inputs):
    global LAST_RESULT
    from concourse.bass_utils import run_bass_kernel_spmd

    cfg = np.asarray(inputs["cfg_indices_padded"])
    hint_state = np.ascontiguousarray(np.asarray(inputs["hint_state"],
                                                 dtype=np.float32))
    node_fts = np.asarray(inputs["node_fts"], dtype=np.float32)
    edge_fts = np.asarray(inputs["edge_fts"], dtype=np.float32)
    edge_W = np.asarray(inputs["edge_W"], dtype=np.float32)
    edge_b = np.asarray(inputs["edge_b"], dtype=np.float32)
    update_W = np.ascontiguousarray(np.asarray(inputs["update_W"],
                                               dtype=np.float32))
    update_b = np.asarray(inputs["update_b"], dtype=np.float32)

    src = np.asarray(cfg[..., 0], dtype=np.int64)
    tgt = np.asarray(cfg[..., 1], dtype=np.int64)

    bf16 = not bool(int(os.environ.get("KERNEL_F32", "0")))
    key = ("nc", RPD, bf16)
    if key not in _CACHE:
        _CACHE[key] = _build(RPD, bf16)
    nc = _CACHE[key]

    if bf16:
        import ml_dtypes
        wdt = ml_dtypes.bfloat16
    else:
        wdt = np.float32

    eW_in = np.ascontiguousarray(edge_W[:, 0].reshape(2, P).T).astype(wdt)
    eb_in = np.full((P, 1), edge_b[0], np.float32)
    ub_in = np.ascontiguousarray(np.tile(update_b, M)[None, :]).astype(wdt)
    uW_in = update_W.astype(wdt)

    in_maps = []
    for b in range(B):
        grid = _edge_grid(tgt[b])             # [N, K]
        srcg = src[b][grid]                   # [N, K]
        hint_b = hint_state[b].reshape(N, MH).astype(wdt)
        for q in range(4):
            g_q = grid[q * NCORE:(q + 1) * NCORE]    # [1024, K]
            s_q = srcg[q * NCORE:(q + 1) * NCORE]
            if RPD == 2:
                trail, gidx, eids_nk = _pair_trails(s_q, g_q)
                nrows = len(trail)
                assert nrows <= TMAX, nrows
                tbl = np.zeros((TMAX, MH), wdt)
                tbl[:nrows] = hint_b[trail]
            else:
                gidx = s_q                    # [1024, 16] source rows
                eids_nk = g_q
                tbl = hint_b
            # gather index order: i = nb*NIDX + g*128 + p
            # -> window g of node nb*128 + p.
            gi = gidx.reshape(NB, P, 2, WINS_PT)       # [nb, p, h, g]
            flat = gi.transpose(0, 2, 3, 1).reshape(-1)  # [nb, h, g, p]
            idx16 = np.ascontiguousarray(
                np.tile(flat.reshape(NWIN_CORE // 16, 16).T, (8, 1))
            ).astype(np.int16)
            # edge-feature column order: j = nb*2048 + p*16 + r (node-major)
            eids = eids_nk.reshape(NB * P * K)
            efts_t = np.ascontiguousarray(edge_fts[b][eids].T).astype(wdt)
            nf_q = np.ascontiguousarray(
                node_fts[b, q * NCORE:(q + 1) * NCORE].reshape(NCORE, MH)
            ).astype(wdt)
            in_maps.append({
                "table": tbl,
                "eftsT": efts_t,
                "idx16": idx16,
                "nf": nf_q,
                "eW": eW_in,
                "eb": eb_in,
                "uW": uW_in,
                "ub": ub_in,
            })

    trace = bool(int(os.environ.get("KERNEL_TRACE", "0")))
    if trace:
        _install_ntff_hook()
    res = run_bass_kernel_spmd(nc, in_maps, core_ids=list(range(N_CORES)),
                               trace=trace)
    if trace:
        LAST_RESULT = res

    out = np.empty((B, N, M, H), np.float32)
    for b in range(B):
        for q in range(4):
            o = np.asarray(res.results[b * 4 + q]["out"], np.float32)
            out[b, q * NCORE:(q + 1) * NCORE] = o.reshape(NCORE, M, H)
    return out
